# revision 37
# baseline (speedup 1.0000x reference)
"""MeanStdMax pooling kernel for Trainium2 (8 NeuronCores, data-parallel).

Input : hidden_states [16, 13, 512, 768] fp32
Output: [16, 13, 2304] fp32 = concat([sum(seq), std(seq, ddof=1), max(seq)], -1)

Sharding: batch dim 16 -> 2 batches per core (no cross-core communication).

Per-core plan (26 (b,l) pairs, each [512, 768] = 1.57MB; HBM floor ~4.3us/pair
measured as a pure 8-core stream):
  - DMA each pair as one [128, 4*768] tile; partition p holds seq rows
    4p..4p+3 (contiguous 12KB DRAM chunks). Pairs 0/1 stream as quarter
    tiles into dedicated (never-reused) tiles so compute starts ~5us in;
    pair 25 streams as two halves so the tail chain starts early.
  - sum  : fp32r one-hot-weight matmuls off the raw tile into PSUM rows.
    Two accumulation groups SHARE banks (group B's start matmuls are
    ordered behind epilogue A's psum reads by the tile tracker).  Each
    accumulator gets a FULL psum bank: a series' start=True matmul wipes
    the whole bank, so two interleaved series must never share one (this
    corrupted the first two pairs of each group when packed).
  - sumsq: ACT Square -> bf16, then bf16 one-hot matmuls (lagged one pair
    so PE never head-of-line blocks on the current pair's squares).
  - max  : all-fp32 DVE tree (bf16 output on DVE costs 2.2x fp32):
    m2[128,2,768] then Mf[128,768].  Partition reduce on gpsimd
    (fp32 partition_all_reduce, ~3.8-4.2us/pair, under the DMA pace) for
    all but the last two pairs, which take PE-transpose + DVE reduce_max
    + PE fixup transpose so gpsimd's serial queue never paces the tail.
    gpsimd results land in wide staging tiles, 4 pairs per output DMA.
  - outputs ride the scalar (ACT) HWDGE ring so the sync ring carries only
    input loads; max-out DMAs lag ~1 group so their deps are long
    resolved (each dma_start costs ~0.6us of sequencer dispatch, hence
    the batching).
  - epilogue: sums copy + sum^2 (ACT Square, scaled) as each group's sum
    psum closes; var subtract on DVE into one wide tile; a single tail
    sqrt over all 26 rows with the Sqrt table preloaded right after the
    last Square (no mid-stream table thrash); one stds DMA.
"""

import os
import sys

import numpy as np

for _p in ("/opt/trn_rl_repo", "/root/.axon_site/_ro/trn_rl_repo"):
    if os.path.isdir(_p) and _p not in sys.path:
        sys.path.insert(0, _p)

import concourse.bacc as bacc
import concourse.bass as bass
import concourse.bass_isa as bass_isa
import concourse.masks as masks
import concourse.mybir as mybir
import concourse.tile as tile
from concourse.bass_utils import run_bass_kernel_spmd

N_CORES = 8
B_FULL, L, S, H = 16, 13, 512, 768
B = B_FULL // N_CORES  # 2 batches per core
P = 128
NBLK = S // P  # 4
NPAIR = B * L  # 26
F32 = mybir.dt.float32
F32R = mybir.dt.float32r
BF16 = mybir.dt.bfloat16

GSPLIT = 18              # stats psum groups: pairs [0,18) and [18,26)
QSPLIT = (0, 1)          # stream these pairs as four quarter-tiles
HSPLIT = (25,)           # stream these pairs as two half-tiles
DEDICATED = (0, 1)       # own (never pool-reused) input tiles
# PE-transpose max path for the LAST pairs (gpsimd's serial queue lags the
# stream by a few pairs, so routing the tail through PE+DVE cuts ~8us of
# post-stream gpsimd drain); the rest go through gpsimd.
PEP = (21, 22, 23, 24, 25)
MGRP = 4                 # gpsimd max results per staging tile / output DMA

_CACHE = {}


def _build():
    if "nc" in _CACHE:
        return _CACHE["nc"]

    nc = bacc.Bacc("TRN2", target_bir_lowering=False, debug=False,
                   num_devices=N_CORES)
    # float32r: same bits as fp32, but satisfies the BIR verifier's
    # "rounded to FP32r" rule so DMA-loaded tiles can feed fp32r matmuls
    # (the fast single-pass fp32 PE mode).
    x = nc.dram_tensor("x", [B, L, S, H], F32R, kind="ExternalInput").ap()
    out = nc.dram_tensor("out", [B, L, 3 * H], F32, kind="ExternalOutput").ap()
    out2 = out.rearrange("b l h -> (b l) h")  # [26, 2304]

    with tile.TileContext(nc) as tc:
        with (
            tc.tile_pool(name="inp", bufs=7) as in_pool,
            tc.tile_pool(name="inp0", bufs=len(DEDICATED)) as in0_pool,
            tc.tile_pool(name="sq", bufs=3) as sq_pool,
            tc.tile_pool(name="m2w", bufs=3) as m2w_pool,
            tc.tile_pool(name="mbig", bufs=2) as mbig_pool,
            tc.tile_pool(name="const", bufs=1) as const_pool,
            tc.tile_pool(name="ep", bufs=1) as ep_pool,
            tc.tile_pool(name="psum", bufs=1, space="PSUM") as psum_pool,
        ):
            # one-hot weight bank: W0[:, NPAIR-j : NPAIR-j+32] is all-ones
            # exactly at local column j.
            W0 = const_pool.tile([P, NPAIR + 32], F32)
            nc.gpsimd.memset(W0[:], 0.0)
            nc.gpsimd.memset(W0[:, NPAIR:NPAIR + 1], 1.0)
            Wr = W0[:].bitcast(F32R)
            Wb = const_pool.tile([P, NPAIR + 32], BF16)
            nc.vector.tensor_copy(Wb[:], W0[:])

            Ident = const_pool.tile([P, P], F32)
            masks.make_identity(nc, Ident[:])

            ps_sum_a = psum_pool.tile([32, 512], F32, name="sum_a", tag="sum_a")
            ps_sq_a = psum_pool.tile([32, 512], F32, name="sq_a", tag="sq_a")
            ps_sum_b = psum_pool.tile([32, 512], F32, name="sum_b", tag="sum_b")
            ps_sq_b = psum_pool.tile([32, 512], F32, name="sq_b", tag="sq_b")
            PS = {
                "sum_a": ps_sum_a[:],
                "sum_b": ps_sum_b[:, 0:256],
                "sq_a": ps_sq_a[:],
                "sq_b": ps_sq_b[:, 0:256],
            }
            # tail max scratch: TP holds a pair's Mf transposed (hidden on
            # partitions); the per-pair DVE reduce_max writes straight into
            # a slot of R30, and ONE tail transpose + copy + DMA emits all
            # PEP pairs' maxes.
            TP = psum_pool.tile([P, H], F32, name="TP", tag="TP")
            TP2 = psum_pool.tile([32, P], F32, name="TP2", tag="TP2")
            R30 = ep_pool.tile([P, 8 * ((6 * len(PEP) + 7) // 8)], F32,
                               name="R30")
            stgT = ep_pool.tile([32, P], F32, name="stgT")

            def grp(j):
                return 0 if j < GSPLIT else 1

            def is_start(j):
                return j in (0, GSPLIT)

            def is_stop(j):
                return j in (GSPLIT - 1, NPAIR - 1)

            def local(j):
                return j - (0 if j < GSPLIT else GSPLIT)

            # var/std epilogue state: wide tiles (group B at partition 32 —
            # engine APs must start quadrant-aligned), single tail sqrt.
            NA, NB = GSPLIT, NPAIR - GSPLIT
            EPN = 32 + NB
            sums_sb = ep_pool.tile([EPN, H], F32, name="sums_sb")
            sum2_sb = ep_pool.tile([EPN, H], F32, name="sum2_sb")
            var_sb = ep_pool.tile([EPN, H], F32, name="var_sb")
            std_sb = ep_pool.tile([EPN, H], F32, name="std_sb")
            sqrt_dummy = ep_pool.tile([1, 1], F32, name="sqrt_dummy")

            def close_sums(g):
                # needs the group's SUM psum closed; Square table loaded.
                lo, n = (0, NA) if g == 0 else (32, NB)
                row = 0 if g == 0 else GSPLIT
                nc.scalar.copy(sums_sb[lo:lo + n, 0:512], PS["sum_a"][0:n])
                nc.scalar.copy(sums_sb[lo:lo + n, 512:768], PS["sum_b"][0:n])
                nc.scalar.activation(sum2_sb[lo:lo + n], sums_sb[lo:lo + n],
                                     mybir.ActivationFunctionType.Square,
                                     scale=1.0 / float(np.sqrt(S)))
                nc.scalar.dma_start(out2[row:row + n, 0:H],
                                    sums_sb[lo:lo + n])

            def close_var(g):
                # needs the group's SQ psum closed (+ sum2 from close_sums).
                lo, n = (0, NA) if g == 0 else (32, NB)
                nc.vector.tensor_tensor(var_sb[lo:lo + n, 0:512],
                                        PS["sq_a"][0:n],
                                        sum2_sb[lo:lo + n, 0:512],
                                        op=mybir.AluOpType.subtract)
                nc.vector.tensor_tensor(var_sb[lo:lo + n, 512:768],
                                        PS["sq_b"][0:n],
                                        sum2_sb[lo:lo + n, 512:768],
                                        op=mybir.AluOpType.subtract)

            # PE runs one pair behind for sq matmuls so its per-iteration
            # work only depends on data from iteration j-1.
            pending = None  # (j, Q_tile)
            # gpsimd max results accumulate MGRP to a staging tile; one
            # output DMA per full group (issued on the NEXT pair, so the
            # last reduce's dep is resolved by then).
            mstage = {"tile": None, "base": None, "cnt": 0}
            pend_mdma = []

            def flush_mstage():
                t = mstage
                if t["cnt"]:
                    src = t["tile"][0:1, 0:t["cnt"] * H].rearrange(
                        "1 (t h) -> 1 t h", h=H)
                    dst = out2[t["base"]:t["base"] + t["cnt"],
                               2 * H:3 * H].rearrange("(o t) h -> o t h", o=1)
                    pend_mdma.append((dst, src))
                    mstage["tile"] = None
                    mstage["cnt"] = 0

            def emit_sq_mm(j, Q):
                g = grp(j)
                first, last = is_start(j), is_stop(j)
                lj = local(j)
                wjb = Wb[:, NPAIR - lj:NPAIR - lj + 32]
                Qv = Q[:].rearrange("p (n h) -> p n h", h=H)
                for blk in range(NBLK):
                    nc.tensor.matmul(
                        PS["sq_a"], wjb, Qv[:, blk, 0:512],
                        start=first and blk == 0, stop=last and blk == NBLK - 1)
                    nc.tensor.matmul(
                        PS["sq_b"], wjb, Qv[:, blk, 512:768],
                        start=first and blk == 0, stop=last and blk == NBLK - 1)

            for j in range(NPAIR):
                b, l = divmod(j, L)
                g = grp(j)
                first, last = is_start(j), is_stop(j)

                if j == GSPLIT:
                    # group A psum must fully retire (last sq matmuls + the
                    # epilogue's psum reads) BEFORE group B's start matmuls
                    # reuse the shared banks.
                    if pending is not None:
                        emit_sq_mm(*pending)
                        pending = None
                    close_sums(0)
                    close_var(0)

                pool = in0_pool if j in DEDICATED else in_pool
                T = pool.tile([P, NBLK * H], F32R)
                Tr = T[:].rearrange("p (n h) -> p n h", h=H)
                # partition p <- seq rows 4p..4p+3: contiguous 12KB chunks;
                # the seq->(p,i) mapping is irrelevant to sum/max/sumsq.
                src = x[b, l].rearrange("(p n) h -> p n h", n=NBLK)
                if j in QSPLIT:
                    for q in range(NBLK):
                        nc.sync.dma_start(Tr[:, q:q + 1, :], src[:, q:q + 1, :])
                elif j in HSPLIT:
                    nc.sync.dma_start(Tr[:, 0:2, :], src[:, 0:2, :])
                    nc.sync.dma_start(Tr[:, 2:4, :], src[:, 2:4, :])
                else:
                    nc.sync.dma_start(T[:], src)
                Tv = T[:].bitcast(F32).rearrange("p (n h) -> p n h", h=H)

                # ---- sums: fp32r one-hot matmuls off the raw tile ----
                lj = local(j)
                wjr = Wr[:, NPAIR - lj:NPAIR - lj + 32]
                for blk in range(NBLK):
                    nc.tensor.matmul(
                        PS["sum_a"], wjr, Tr[:, blk, 0:512],
                        start=first and blk == 0, stop=last and blk == NBLK - 1)
                    nc.tensor.matmul(
                        PS["sum_b"], wjr, Tr[:, blk, 512:768],
                        start=first and blk == 0, stop=last and blk == NBLK - 1)

                # ---- max tree on DVE, all fp32.  Level-1 results land in
                # ONE wide tile with the halves 6KB apart: DVE dual-read
                # throughput is SBUF-bank-phase dependent (measured 1.1-1.4
                # ns/col same-tile at 3/6KB separation vs 2.2-2.9 across
                # pool tiles), and same-tile @6KB is the reliably fast
                # pattern (matches the level-1 reads of the raw tile). ----
                m2w = m2w_pool.tile([P, NBLK * H], F32, tag="m2w")
                m2a = m2w[:, 0:H]
                m2b = m2w[:, 2 * H:3 * H]
                Mf = m2w[:, 3 * H:4 * H]
                # incremental CHAIN fold (not a tree): each op reads one
                # fresh DVE result + one DMA-loaded block.  Measured: ops
                # whose two source streams are BOTH engine-written run at
                # 2.9 ns/col vs ~1.25 with at least one DMA-written source,
                # so the chain (3x ~960ns) beats the tree (958+868+2232).
                # Also: only the last op depends on the final quarter/half.
                nc.vector.tensor_tensor(m2a, Tv[:, 0, :], Tv[:, 1, :],
                                        op=mybir.AluOpType.max)
                nc.vector.tensor_tensor(m2b, m2a, Tv[:, 2, :],
                                        op=mybir.AluOpType.max)
                nc.vector.tensor_tensor(Mf, m2b, Tv[:, 3, :],
                                        op=mybir.AluOpType.max)

                if j not in PEP:
                    # fp32 partition all-reduce on gpsimd into the staging
                    # tile's column slot; one DMA per MGRP pairs.
                    if mstage["cnt"] == 0:
                        mstage["tile"] = mbig_pool.tile([P, MGRP * H], F32,
                                                        name="Mbig",
                                                        tag="Mbig")
                        mstage["base"] = j
                    c = mstage["cnt"]
                    nc.gpsimd.partition_all_reduce(
                        mstage["tile"][:, c * H:(c + 1) * H], Mf,
                        channels=P, reduce_op=bass_isa.ReduceOp.max)
                    mstage["cnt"] = c + 1
                    if mstage["cnt"] == MGRP:
                        flush_mstage()
                else:
                    # PE transposes + DVE reduce_max into this pair's R30
                    # slot; one shared tail transpose emits all PEP maxes.
                    t = j - PEP[0]
                    for c in range(6):
                        nc.tensor.transpose(TP[:, P * c:P * (c + 1)],
                                            Mf[:, P * c:P * (c + 1)],
                                            Ident[:])
                    nc.vector.reduce_max(
                        R30[:, 6 * t:6 * t + 6],
                        TP[:].rearrange("p (c e) -> p c e", e=P),
                        axis=mybir.AxisListType.X)

                # ---- squares in bf16 on ACT ----
                Q = sq_pool.tile([P, NBLK * H], BF16)
                if j in QSPLIT:
                    for q in range(NBLK):
                        nc.scalar.activation(
                            Q[:, q * H:(q + 1) * H],
                            T[:, q * H:(q + 1) * H].bitcast(F32),
                            mybir.ActivationFunctionType.Square)
                elif j in HSPLIT:
                    nc.scalar.activation(Q[:, 0:2 * H],
                                         T[:, 0:2 * H].bitcast(F32),
                                         mybir.ActivationFunctionType.Square)
                    nc.scalar.activation(Q[:, 2 * H:4 * H],
                                         T[:, 2 * H:4 * H].bitcast(F32),
                                         mybir.ActivationFunctionType.Square)
                else:
                    nc.scalar.activation(Q[:], T[:].bitcast(F32),
                                         mybir.ActivationFunctionType.Square)

                if pending is not None:
                    emit_sq_mm(*pending)
                pending = (j, Q)
                if j == NPAIR - 1:
                    # last pair: don't lag, PE finishes right after squares
                    emit_sq_mm(*pending)
                    pending = None
                    # group B sums psum closed at this pair's sum matmuls;
                    # run its ACT pieces while the Square table is loaded,
                    # THEN preload the Sqrt table.
                    close_sums(1)
                    nc.scalar.sqrt(sqrt_dummy[:], sqrt_dummy[:])
                # issue at most one pending max-out DMA per pair; by issue
                # time its reduces are long done.  The last groups are
                # dispatched from the sync sequencer in the tail instead
                # (their gpsimd deps may still be pending, and the wait
                # would head-of-line block ACT's tail chain).
                if len(pend_mdma) > 2 and j < PEP[0]:
                    dst, src2 = pend_mdma.pop(0)
                    nc.scalar.dma_start(dst, src2)

            # ---- tail ----
            flush_mstage()
            close_var(1)
            # PEP maxes: one layout-fix transpose of all R30 slots, DVE
            # copy out of psum, one DMA.
            NT = 6 * len(PEP)
            nc.tensor.transpose(TP2[0:NT, :], R30[:, 0:NT], Ident[:])
            nc.vector.tensor_copy(stgT[0:NT, :], TP2[0:NT, :])
            for t in range(len(PEP)):
                nc.sync.dma_start(
                    out2[PEP[0] + t:PEP[0] + t + 1, 2 * H:3 * H].rearrange(
                        "1 (c e) -> c e", e=P),
                    stgT[6 * t:6 * t + 6, :])
            # remaining gpsimd max-out DMAs dispatch from the (now idle)
            # SYNC sequencer so they never head-of-line block the ACT sqrt
            # chain; their packets land on Q_I after all input packets.
            for dst, src2 in pend_mdma:
                nc.sync.dma_start(dst, src2)
            pend_mdma.clear()
            # tail sqrts for both groups (table already loaded)
            for lo, n, row in ((0, NA, 0), (32, NB, GSPLIT)):
                nc.scalar.activation(std_sb[lo:lo + n], var_sb[lo:lo + n],
                                     mybir.ActivationFunctionType.Sqrt,
                                     scale=1.0 / (S - 1))
                nc.scalar.dma_start(out2[row:row + n, H:2 * H],
                                    std_sb[lo:lo + n])

    nc.compile()
    _CACHE["nc"] = nc
    return nc


def _run(hidden_states: np.ndarray, trace: bool = False):
    nc = _build()
    x = np.ascontiguousarray(np.asarray(hidden_states, dtype=np.float32))
    assert x.shape == (B_FULL, L, S, H), x.shape
    in_maps = [{"x": x[c * B:(c + 1) * B]} for c in range(N_CORES)]
    res = run_bass_kernel_spmd(nc, in_maps, core_ids=list(range(N_CORES)),
                               trace=trace)
    out = np.empty((B_FULL, L, 3 * H), dtype=np.float32)
    for c in range(N_CORES):
        out[c * B:(c + 1) * B] = res.results[c]["out"]
    return out, res


def kernel(hidden_states: np.ndarray) -> np.ndarray:
    out, _ = _run(hidden_states)
    return out
